# revision 11
# baseline (speedup 1.0000x reference)
"""1-NN lookup (nearest-neighbor interpolation) Bass kernel for Trainium2.

Problem: points_q [16384,2], points [8192,2], values [8192] (all fp32).
out[q] = values[argmin_j d2(q,j)] with d2 computed exactly like the jax
reference: d2 = (qn - 2*(q@p.T)) + pn, argmin = first index of the min.

Sharding: queries split across 8 NeuronCores (2048 each); points/values
replicated. No collectives.

Per core, bitwise-matching the op-by-op jax-on-neuron reference:
  - PE fp32 matmul K=2 computes qp tiles (identical instruction to XLA's dot).
  - ACT computes u = fl(-2*qp + qn) (Identity activation, per-partition bias).
  - GPSIMD computes neg_d2 = fl(-pn - u) = -d2 (IEEE fp32 subtract).
  - DVE max8 + max_index give the first-occurrence argmax of -d2 == argmin d2.
  - Final gather values[idx] runs on-device via exact one-hot matmuls
    (idx = 64*hi + lo; onehot_hi^T @ values2d selects a 64-row; onehot_lo
    dot-selects within it; all products are 0/1*v so fp32-exact).
"""

import numpy as np
import concourse.bass as bass
import concourse.mybir as mybir
from concourse.bass_utils import run_bass_kernel_spmd

dt = mybir.dt

NCORES = 8
N = 16384
NSHARD = N // NCORES   # 2048 queries per core
M = 8192               # points
P = 128                # partitions / queries per block
B = NSHARD // P        # 16 query blocks
TW = 512               # point-tile width
T = M // TW            # 16 point tiles per block
NBANKS = 8             # psum ring for score tiles
URING = 16             # u_sb ring tiles

_CACHE = {}


def _build():
    nc = bass.Bass()
    PQ = nc.declare_dram_parameter("PQ", [NSHARD, 2], dt.float32, isOutput=False)
    PQT = nc.declare_dram_parameter("PQT", [2, NSHARD], dt.float32, isOutput=False)
    PTST = nc.declare_dram_parameter("PTST", [2, M], dt.float32, isOutput=False)
    VALS = nc.declare_dram_parameter("VALS", [M], dt.float32, isOutput=False)
    IOTA = nc.declare_dram_parameter("IOTA", [P, P], dt.float32, isOutput=False)
    IDENT = nc.declare_dram_parameter("IDENT", [P, P], dt.float32, isOutput=False)
    OUT = nc.declare_dram_parameter("OUT", [NSHARD], dt.float32, isOutput=True)

    from contextlib import ExitStack
    with ExitStack() as ctx:
        def sb(name, shape, dtype=dt.float32):
            return ctx.enter_context(nc.sbuf_tensor(name, shape, dtype))

        # big SBUF residents
        qt = sb("qt", [2, NSHARD])          # [qx;qy] transposed
        pt = sb("pt", [2, M])               # [px;py] transposed
        npnb = sb("npnb", [P, M])           # -pn broadcast to 128 rows
        u_sb = sb("u_sb", [P, URING * TW])  # fl(-2qp+qn) ring
        nega = sb("nega", [P, M])           # -d2, block buffer A
        negb = sb("negb", [P, M])           # -d2, block buffer B
        # smalls
        qraw = sb("qraw", [P, 2 * B])
        qsq = sb("qsq", [P, 2 * B])
        qn_t = sb("qn_t", [P, B])
        ones_r = sb("ones_r", [1, P])
        m8 = sb("m8", [P, 8])
        i8 = sb("i8", [P, 8], dt.uint32)
        idx_all = sb("idx_all", [P, B], dt.uint32)
        hi_u = sb("hi_u", [P, B], dt.uint32)
        lo_u = sb("lo_u", [P, B], dt.uint32)
        hi_f = sb("hi_f", [P, B])
        lo_f = sb("lo_f", [P, B])
        vals2d = sb("vals2d", [P, 64])
        iota = sb("iota", [P, P])
        ident = sb("ident", [P, P])
        oh_hi = sb("oh_hi", [P, P])
        oh_lo = sb("oh_lo", [P, 64])
        ohT = sb("ohT", [P, P])
        scr = sb("scr", [P, 64])
        outv = sb("outv", [P, B])
        out_sb = sb("out_sb", [B, P])
        # psum: 8 banks of 512 fp32
        ps = ctx.enter_context(nc.psum_tensor("ps", [P, NBANKS * TW], dt.float32))
        s_dma = ctx.enter_context(nc.semaphore("s_dma"))
        s_mm = ctx.enter_context(nc.semaphore("s_mm"))
        s_act = ctx.enter_context(nc.semaphore("s_act"))
        s_gp = ctx.enter_context(nc.semaphore("s_gp"))
        s_dv = ctx.enter_context(nc.semaphore("s_dv"))
        block = ctx.enter_context(nc.Block())
        negbuf = [nega, negb]
        # partition-0 M-wide scratch rows inside not-yet-used big tensors
        # (all engine accesses must start at partition 0/32/64)
        rx = u_sb[0:1, 0:M]       # px row (DMA)
        ry = nega[0:1, 0:M]       # py row (DMA)
        sqx = npnb[0:1, 0:M]      # px^2
        sqy = negb[0:1, 0:M]      # py^2, later reused for -pn
        pn_row = u_sb[0:1, 0:M]   # px^2+py^2 (overwrites rx, fine)
        npn_row = negb[0:1, 0:M]  # -pn (overwrites sqy, fine)

        # ---- semaphore count bookkeeping (python-side mirrors) ----
        # s_dma: input DMAs, 6 x 16 = 96
        # s_dv : setup 6; main 3/block; final 4 + 4/block
        # s_mm : setup 16 (bcast); main 256; final 2/block
        # s_act: setup 16 (bcast copies); main 256; final 1/block
        # s_gp : main 256
        DV_SETUP = 7
        DV_MAIN = DV_SETUP + 3 * B
        MM_SETUP = T                          # 16
        MM_MAIN = MM_SETUP + B * T            # 272
        ACT_SETUP = T
        ACT_MAIN = ACT_SETUP + B * T          # 272
        DV_F0 = DV_MAIN + 4                   # 58: after hi/lo extract

        @block.sync
        def _(sync):
            sync.dma_start(qt[:], PQT[:]).then_inc(s_dma, 16)
            sync.dma_start(pt[:], PTST[:]).then_inc(s_dma, 16)
            sync.dma_start(
                qraw[:].rearrange("p (b k) -> p b k", k=2),
                PQ[:].rearrange("(b p) k -> p b k", p=P),
            ).then_inc(s_dma, 16)
            sync.dma_start(
                vals2d[:], VALS[:].rearrange("(h l) -> h l", h=P)
            ).then_inc(s_dma, 16)
            sync.dma_start(iota[:], IOTA[:]).then_inc(s_dma, 16)
            sync.dma_start(ident[:], IDENT[:]).then_inc(s_dma, 16)
            sync.dma_start(rx, PTST[0:1, :]).then_inc(s_dma, 16)
            sync.dma_start(ry, PTST[1:2, :]).then_inc(s_dma, 16)
            # final output (contiguous, after on-device transpose)
            sync.wait_ge(s_act, ACT_MAIN + B + 1)
            sync.dma_start(
                OUT[:].rearrange("(b p) -> b p", p=P), out_sb[:]
            ).then_inc(s_dma, 16)

        @block.vector
        def _(vector):
            # ---- setup ----
            vector.memset(ones_r[:], 1.0).then_inc(s_dv, 1)    # dv=1
            vector.wait_ge(s_dma, 128)
            vector.tensor_tensor(qsq[:], qraw[:], qraw[:], op=mybir.AluOpType.mult).then_inc(s_dv, 1)  # 2
            vector.tensor_tensor(sqx, rx, rx, op=mybir.AluOpType.mult).then_inc(s_dv, 1)               # 3
            vector.tensor_tensor(sqy, ry, ry, op=mybir.AluOpType.mult).then_inc(s_dv, 1)               # 4
            vector.wait_ge(s_dv, 4)
            vector.tensor_reduce(
                qn_t[:], qsq[:].rearrange("p (b k) -> p b k", k=2),
                axis=mybir.AxisListType.X, op=mybir.AluOpType.add,
            )
            vector.drain().then_inc(s_dv, 1)                   # 5 (commit qn_t)
            vector.tensor_tensor(pn_row, sqx, sqy, op=mybir.AluOpType.add).then_inc(s_dv, 1)  # 6
            vector.wait_ge(s_dv, 6)
            vector.tensor_scalar_mul(npn_row, pn_row, -1.0)
            vector.drain().then_inc(s_dv, 1)                   # 7 == DV_SETUP (commit npn_row)

            # ---- main loop: per block argmax of -d2 ----
            dv = DV_SETUP
            for b in range(B):
                vector.wait_ge(s_gp, T * (b + 1))
                vector.max(m8[:], negbuf[b % 2][:]).then_inc(s_dv, 1); dv += 1
                vector.wait_ge(s_dv, dv)
                vector.max_index(i8[:], m8[:], negbuf[b % 2][:]).then_inc(s_dv, 1); dv += 1
                vector.wait_ge(s_dv, dv)
                vector.tensor_copy(idx_all[:, b:b + 1], i8[:, 0:1]).then_inc(s_dv, 1); dv += 1
            assert dv == DV_MAIN

            # ---- final: values[idx] via exact one-hot matmuls ----
            vector.wait_ge(s_dv, DV_MAIN)   # idx_all col B-1 committed
            vector.drain()
            vector.tensor_scalar(hi_u[:], idx_all[:], 6, None,
                                 op0=mybir.AluOpType.logical_shift_right).then_inc(s_dv, 1); dv += 1
            vector.tensor_scalar(lo_u[:], idx_all[:], 63, None,
                                 op0=mybir.AluOpType.bitwise_and).then_inc(s_dv, 1); dv += 1
            vector.wait_ge(s_dv, dv)
            vector.tensor_copy(hi_f[:], hi_u[:]).then_inc(s_dv, 1); dv += 1
            vector.tensor_copy(lo_f[:], lo_u[:]).then_inc(s_dv, 1); dv += 1
            assert dv == DV_F0
            for b in range(B):
                vector.wait_ge(s_dv, dv)
                vector.tensor_scalar(oh_hi[:], iota[:], hi_f[:, b:b + 1], None,
                                     op0=mybir.AluOpType.is_equal)
                vector.drain().then_inc(s_dv, 1)               # commit oh_hi
                vector.tensor_scalar(oh_lo[:], iota[:, 0:64], lo_f[:, b:b + 1], None,
                                     op0=mybir.AluOpType.is_equal).then_inc(s_dv, 1)
                dv += 2
                vector.wait_ge(s_mm, MM_MAIN + 2 * b + 2)   # U matmul done
                vector.tensor_tensor(scr[:], ps[:, TW:TW + 64], oh_lo[:],
                                     op=mybir.AluOpType.mult).then_inc(s_dv, 1); dv += 1
                vector.wait_ge(s_dv, dv)
                vector.tensor_reduce(outv[:, b:b + 1], scr[:],
                                     axis=mybir.AxisListType.X,
                                     op=mybir.AluOpType.add).then_inc(s_dv, 1); dv += 1
            # commit outv before the PE output transpose reads it
            vector.wait_ge(s_dv, dv)
            vector.drain().then_inc(s_dv, 1); dv += 1

        @block.tensor
        def _(tensor):
            # ---- setup: broadcast -pn to 128 partitions (exact 1*x matmul) ----
            tensor.wait_ge(s_dv, DV_SETUP)
            mm = 0
            for j in range(T):
                if j >= NBANKS:
                    tensor.wait_ge(s_act, j - NBANKS + 1)
                tensor.matmul(ps[:, (j % NBANKS) * TW:(j % NBANKS + 1) * TW],
                              ones_r[:], npn_row[:, j * TW:(j + 1) * TW],
                              start=True, stop=True).then_inc(s_mm, 1)
                mm += 1
            # ---- main: qp tiles ----
            for b in range(B):
                for t in range(T):
                    gt = b * T + t
                    if gt >= NBANKS:
                        # bank reuse: ACT must have drained tile gt-8
                        tensor.wait_ge(s_act, ACT_SETUP + gt - NBANKS + 1)
                    tensor.matmul(ps[:, (gt % NBANKS) * TW:(gt % NBANKS + 1) * TW],
                                  qt[:, b * P:(b + 1) * P],
                                  pt[:, t * TW:(t + 1) * TW],
                                  start=True, stop=True).then_inc(s_mm, 1)
                    mm += 1
            assert mm == B * T + T
            # ---- final: transpose + U matmuls (banks 0 and 1) ----
            tensor.wait_ge(s_act, ACT_MAIN)  # all psum drained
            for b in range(B):
                tensor.wait_ge(s_dv, DV_F0 + 4 * b + 1)      # oh_hi ready
                tensor.transpose(ps[:, 0:P], oh_hi[:], ident[:]).then_inc(s_mm, 1)
                tensor.wait_ge(s_act, ACT_MAIN + b + 1)       # ohT copied
                tensor.matmul(ps[:, TW:TW + 64], ohT[:], vals2d[:],
                              start=True, stop=True).then_inc(s_mm, 1)
            # transpose outv -> [B, P] for a contiguous output DMA
            tensor.wait_ge(s_dv, DV_F0 + 4 * B + 1)
            tensor.transpose(ps[0:B, 0:P], outv[:], ident[:]).then_inc(s_mm, 1)

        @block.scalar
        def _(scalar):
            # ---- setup: copy -pn broadcast tiles to SBUF ----
            for j in range(T):
                scalar.wait_ge(s_mm, j + 1)
                scalar.copy(npnb[:, j * TW:(j + 1) * TW],
                            ps[:, (j % NBANKS) * TW:(j % NBANKS + 1) * TW]).then_inc(s_act, 1)
            # ---- main: u = fl(-2*qp + qn) ----
            for b in range(B):
                for t in range(T):
                    gt = b * T + t
                    scalar.wait_ge(s_mm, MM_SETUP + gt + 1)
                    if gt >= URING:
                        scalar.wait_ge(s_gp, gt - URING + 1)
                    scalar.activation(
                        u_sb[:, (gt % URING) * TW:(gt % URING + 1) * TW],
                        ps[:, (gt % NBANKS) * TW:(gt % NBANKS + 1) * TW],
                        mybir.ActivationFunctionType.Identity,
                        bias=qn_t[:, b:b + 1], scale=-2.0,
                    ).then_inc(s_act, 1)
            # ---- final: ohT copies ----
            for b in range(B):
                scalar.wait_ge(s_mm, MM_MAIN + 2 * b + 1)
                scalar.copy(ohT[:], ps[:, 0:P])
                scalar.drain().then_inc(s_act, 1)              # commit ohT
            scalar.wait_ge(s_mm, MM_MAIN + 2 * B + 1)
            scalar.copy(out_sb[:], ps[0:B, 0:P])
            scalar.drain().then_inc(s_act, 1)                  # commit out_sb

        @block.gpsimd
        def _(gpsimd):
            # ---- main: neg_d2 = fl(-pn - u) ----
            for b in range(B):
                if b >= 2:
                    # negbuf[b%2] reuse: DVE must be done with block b-2
                    gpsimd.wait_ge(s_dv, DV_SETUP + 3 * (b - 1))
                for t in range(T):
                    gt = b * T + t
                    gpsimd.wait_ge(s_act, ACT_SETUP + gt + 1)
                    gpsimd.tensor_tensor(
                        negbuf[b % 2][:, t * TW:(t + 1) * TW],
                        npnb[:, t * TW:(t + 1) * TW],
                        u_sb[:, (gt % URING) * TW:(gt % URING + 1) * TW],
                        op=mybir.AluOpType.subtract,
                    ).then_inc(s_gp, 1)

    return nc


def _get_nc():
    if "nc" not in _CACHE:
        _CACHE["nc"] = _build()
    return _CACHE["nc"]


def _const_inputs():
    iota = np.broadcast_to(
        np.arange(P, dtype=np.float32)[None, :], (P, P)
    ).copy()
    ident = np.eye(P, dtype=np.float32)
    return iota, ident


def make_in_maps(points_q, points, values):
    points_q = np.ascontiguousarray(np.asarray(points_q, dtype=np.float32))
    points = np.ascontiguousarray(np.asarray(points, dtype=np.float32))
    values = np.ascontiguousarray(np.asarray(values, dtype=np.float32))
    assert points_q.shape == (N, 2) and points.shape == (M, 2) and values.shape == (M,)
    iota, ident = _const_inputs()
    ptst = np.ascontiguousarray(points.T)
    in_maps = []
    for c in range(NCORES):
        shard = points_q[c * NSHARD:(c + 1) * NSHARD]
        in_maps.append({
            "PQ": shard,
            "PQT": np.ascontiguousarray(shard.T),
            "PTST": ptst,
            "VALS": values,
            "IOTA": iota,
            "IDENT": ident,
        })
    return in_maps


def kernel(points_q, points, values):
    nc = _get_nc()
    in_maps = make_in_maps(points_q, points, values)
    res = run_bass_kernel_spmd(nc, in_maps, list(range(NCORES)), trace=False)
    out = np.concatenate([res.results[c]["OUT"] for c in range(NCORES)])
    return np.ascontiguousarray(out.astype(np.float32))
